# revision 10
# baseline (speedup 1.0000x reference)
"""Multi-head attention (B=2, S=2048, D=1024, H=16, RoPE) on 8 Trainium2 cores.

Sharding: tensor-parallel over heads. Core c owns heads (2c, 2c+1):
 - W_qkv column-sliced to that head pair (q|k|v blocks of 128 cols each),
 - W_out row-sliced to the pair's 128 input dims,
 - every core reads all tokens (x shipped pre-transposed as x^T),
 - each core emits a partial [4096, 1024] output; host sums the 8 partials
   and adds b_out (the Megatron-style allreduce done on host).

Device program (per core, identical SPMD):
  Phase A (128x128 PE mode): qkv^T = W_c^T @ x^T in fp32r, bias via a
    ones-row matmul, RoPE = (P2^T q^T) * sin + q^T * cos where P2 is the
    rotate-half +/-1 permutation (built on device), V transposed via PE
    into [t, d] layout with an appended ones column.
  Phase B (64x128 row-tiled PE): per (batch, 512-query chunk): for each
    128-key block: scores^T for both heads concurrently (head A on PE
    rows 0-63, head B on rows 64-127), exp on ACT (1/8 scale folded in),
    attn@V accumulated in PSUM with the ones column yielding softmax
    denominators for free; then merge/divide (reciprocal + broadcast
    matmul) and the output projection.
"""

import sys

if "/opt/trn_rl_repo" not in sys.path:
    sys.path.insert(0, "/opt/trn_rl_repo")

import numpy as np

import concourse.bacc as bacc
import concourse.mybir as mybir
from concourse import masks
from concourse.tile import TileContext
from concourse.bass_utils import run_bass_kernel_spmd

F32 = mybir.dt.float32
F32R = mybir.dt.float32r
ADD = mybir.AluOpType.add
MUL = mybir.AluOpType.mult
EXP = mybir.ActivationFunctionType.Exp

B, S, D, H, DH = 2, 2048, 1024, 16, 64
S2 = B * S              # 4096 tokens total
CH = 512                # Phase-A token chunk
NCH = S2 // CH          # 16 chunks
CPB = S // CH           # 8 chunks per batch
NSC = 4                 # 512-query chunks per batch
NTB = S // 128          # 16 key blocks per batch
VG = 130                # V2 group width: dA(64) | 1 | dB(64) | 1


def _build_program():
    nc = bacc.Bacc("TRN2", target_bir_lowering=False, debug=False, num_devices=8)

    xT = nc.dram_tensor("xT", [D, S2], F32, kind="ExternalInput")
    W = nc.dram_tensor("W", [D, 384], F32, kind="ExternalInput")
    bq = nc.dram_tensor("bq", [384], F32, kind="ExternalInput")
    Wo = nc.dram_tensor("Wo", [128, 2048], F32, kind="ExternalInput")
    ctab_d = nc.dram_tensor("ctab", [128, S], F32, kind="ExternalInput")
    stab_d = nc.dram_tensor("stab", [128, S], F32, kind="ExternalInput")
    out_d = nc.dram_tensor("out", [S2, D], F32, kind="ExternalOutput")

    xT_re = xT.rearrange("(kb p) n -> p kb n", p=128)   # [128, 8, 4096]
    W_re = W.rearrange("(kb p) m -> p kb m", p=128)     # [128, 8, 384]

    with TileContext(nc) as tc:
        with tc.tile_pool(name="consts", bufs=1) as cp:
            ident = cp.tile([128, 128], F32, tag="ident")
            masks.make_identity(nc, ident[:])

            P2r = cp.tile([128, 128], F32R, tag="P2r")
            ones_r = cp.tile([128, CH], F32R, tag="ones_r")     # row 0 = 1
            onesv = cp.tile([128, 64], F32, tag="onesv")        # all ones
            bq_r = cp.tile([128, 384], F32R, tag="bq_r")        # row 0 = bias
            ctab = cp.tile([128, S], F32, tag="ctab")
            stab = cp.tile([128, S], F32, tag="stab")
            W_r = cp.tile([128, 8 * 384], F32R, tag="W_r")
            Wo_r = cp.tile([128, 2048], F32R, tag="Wo_r")
            qT = cp.tile([128, S2], F32R, tag="qT")
            kT2a = cp.tile([128, S2], F32R, tag="kT2a")  # rows 64:128 zero
            kT2b = cp.tile([128, S2], F32R, tag="kT2b")  # rows 0:64 zero
            V2 = cp.tile([128, 2 * NTB * VG], F32R, tag="V2")

            nc.sync.dma_start(out=ctab[:], in_=ctab_d[:])
            nc.sync.dma_start(out=stab[:], in_=stab_d[:])

            with tc.tile_pool(name="staging", bufs=1) as sp:
                # rotate-half matrix: P2[k, k^32] = -1 if (k%64)>=32 else +1
                p2f = sp.tile([128, 128], F32, tag="p2f")
                nc.gpsimd.memset(p2f[:], 0.0)
                for bk in (0, 64):
                    nc.gpsimd.affine_select(
                        out=p2f[bk:bk + 32, :], in_=p2f[bk:bk + 32, :],
                        compare_op=mybir.AluOpType.not_equal, fill=1.0,
                        base=bk + 32, channel_multiplier=1, pattern=[[-1, 128]])
                    nc.gpsimd.affine_select(
                        out=p2f[bk + 32:bk + 64, :], in_=p2f[bk + 32:bk + 64, :],
                        compare_op=mybir.AluOpType.not_equal, fill=-1.0,
                        base=bk, channel_multiplier=1, pattern=[[-1, 128]])
                nc.vector.tensor_copy(P2r[:], p2f[:])

                onesf = sp.tile([128, CH], F32, tag="onesf")
                nc.gpsimd.memset(onesf[:], 0.0)
                nc.gpsimd.memset(onesf[0:1, :], 1.0)
                nc.vector.tensor_copy(ones_r[:], onesf[:])

                nc.gpsimd.memset(onesv[:], 1.0)

                bqf = sp.tile([128, 384], F32, tag="bqf")
                nc.gpsimd.memset(bqf[:], 0.0)
                nc.sync.dma_start(out=bqf[0:1, :], in_=bq[None, :])
                nc.vector.tensor_copy(bq_r[:], bqf[:])

                wf = sp.tile([128, 8 * 384], F32, tag="wf")
                nc.sync.dma_start(
                    out=wf[:].rearrange("p (kb m) -> p kb m", kb=8), in_=W_re[:])
                nc.vector.tensor_copy(W_r[:], wf[:])

                wof = sp.tile([128, 2048], F32, tag="wof")
                nc.sync.dma_start(out=wof[:], in_=Wo[:])
                nc.vector.tensor_copy(Wo_r[:], wof[:])

                # zero halves of the padded K^T tiles
                zf = sp.tile([128, 512], F32, tag="zf")
                nc.gpsimd.memset(zf[:], 0.0)
                for i in range(8):
                    nc.vector.tensor_copy(
                        kT2a[64:128, i * 512:(i + 1) * 512], zf[64:128, :])
                    nc.vector.tensor_copy(
                        kT2b[0:64, i * 512:(i + 1) * 512], zf[0:64, :])

            # ones columns of V2 (cols 64 and 129 of each group)
            v2ones = V2[:].rearrange("p (g h r) -> p g h r", g=2 * NTB, h=2)
            nc.vector.tensor_copy(
                v2ones[:, :, :, 64:65],
                onesv[:].rearrange("p (g h r) -> p g h r", g=2 * NTB, h=2))

            # ---------------- Phase A: qkv + rope + V transpose ----------
            with tc.tile_pool(name="xc", bufs=2) as xcp, \
                 tc.tile_pool(name="xr", bufs=2) as xrp, \
                 tc.tile_pool(name="pre", bufs=4) as prep, \
                 tc.tile_pool(name="tmp", bufs=6) as tmpp, \
                 tc.tile_pool(name="vtc", bufs=2) as vtcp, \
                 tc.tile_pool(name="psqkv", bufs=4, space="PSUM") as psqkv, \
                 tc.tile_pool(name="psrot", bufs=2, space="PSUM") as psrot, \
                 tc.tile_pool(name="psv2", bufs=2, space="PSUM") as psv2:
                for ch in range(NCH):
                    cb = ch % CPB           # chunk index within batch
                    scol = cb * CH          # s-offset within batch
                    xc = xcp.tile([128, 8 * CH], F32, tag="xc")
                    nc.sync.dma_start(
                        out=xc[:].rearrange("p (kb n) -> p kb n", kb=8),
                        in_=xT_re[:, :, ch * CH:(ch + 1) * CH])
                    xr = xrp.tile([128, 8 * CH], F32R, tag="xr")
                    nc.vector.tensor_copy(xr[:], xc[:])

                    ps3 = []
                    for mt in range(3):     # q, k, v
                        ps = psqkv.tile([128, CH], F32, tag="qkv")
                        for kb in range(8):
                            nc.tensor.matmul(
                                ps[:],
                                W_r[:, kb * 384 + mt * 128:kb * 384 + (mt + 1) * 128],
                                xr[:, kb * CH:(kb + 1) * CH],
                                start=(kb == 0), stop=False)
                        nc.tensor.matmul(      # bias: bq_pad^T @ ones-row
                            ps[:], bq_r[:, mt * 128:(mt + 1) * 128], ones_r[:],
                            start=False, stop=True)
                        ps3.append(ps)

                    # rope for q and k
                    for mt, dst in ((0, qT), (1, None)):
                        pre = prep.tile([128, CH], F32R, tag="pre")
                        nc.vector.tensor_copy(pre[:], ps3[mt][:])
                        rot = psrot.tile([128, CH], F32, tag="rot")
                        nc.tensor.matmul(rot[:], P2r[:], pre[:], start=True, stop=True)
                        t1 = tmpp.tile([128, CH], F32, tag="tmp")
                        nc.vector.tensor_tensor(
                            out=t1[:], in0=rot[:],
                            in1=stab[:, scol:scol + CH], op=MUL)
                        t2 = tmpp.tile([128, CH], F32, tag="tmp")
                        nc.vector.tensor_tensor(
                            out=t2[:], in0=pre[:],
                            in1=ctab[:, scol:scol + CH], op=MUL)
                        if dst is not None:
                            nc.vector.tensor_tensor(
                                out=dst[:, ch * CH:(ch + 1) * CH],
                                in0=t1[:], in1=t2[:], op=ADD)
                        else:
                            nc.vector.tensor_tensor(
                                out=kT2a[0:64, ch * CH:(ch + 1) * CH],
                                in0=t1[0:64, :], in1=t2[0:64, :], op=ADD)
                            nc.vector.tensor_tensor(
                                out=kT2b[64:128, ch * CH:(ch + 1) * CH],
                                in0=t1[64:128, :], in1=t2[64:128, :], op=ADD)

                    # V: evac + PE transpose into [t, d] with ones cols
                    vt = vtcp.tile([128, CH], F32, tag="vt")
                    nc.vector.tensor_copy(vt[:], ps3[2][:])
                    nt = CH // 128
                    v2p = psv2.tile([128, CH], F32, tag="v2p")
                    for i in range(nt):
                        nc.tensor.transpose(
                            v2p[:, i * 128:(i + 1) * 128],
                            vt[:, i * 128:(i + 1) * 128], ident[:])
                    g0 = (ch // CPB) * NTB + cb * nt
                    dst = V2[:, g0 * VG:(g0 + nt) * VG].rearrange(
                        "p (i h r) -> p i h r", i=nt, h=2)[:, :, :, 0:64]
                    nc.vector.tensor_copy(
                        dst, v2p[:].rearrange("p (i h d) -> p i h d", i=nt, h=2))

            # ---------------- Phase B: attention + out projection --------
            with tc.tile_pool(name="pt", bufs=4) as ptp, \
                 tc.tile_pool(name="mrgc", bufs=2) as mrgc, \
                 tc.tile_pool(name="mrgs", bufs=4) as mrgs, \
                 tc.tile_pool(name="lt", bufs=4) as ltp, \
                 tc.tile_pool(name="a2t", bufs=4) as a2tp, \
                 tc.tile_pool(name="osb", bufs=2) as osbp, \
                 tc.tile_pool(name="dram", bufs=4, space="DRAM") as drp, \
                 tc.tile_pool(name="psscore", bufs=2, space="PSUM") as pssc, \
                 tc.tile_pool(name="psacc", bufs=2, space="PSUM") as psacc, \
                 tc.tile_pool(name="psmisc", bufs=2, space="PSUM") as psmisc:
                for b in range(B):
                    for sc in range(NSC):
                        qcol = b * S + sc * 512
                        gam = [psacc.tile([128, 512], F32, tag="acc",
                                          name=f"gam{i}")
                               for i in range(2)]

                        def av_mms(tb, pa):
                            gcol = (b * NTB + tb) * VG
                            for h in range(2):
                                nc.tensor.matmul(
                                    gam[h][0:65, :],
                                    V2[:, gcol + 65 * h:gcol + 65 * h + 65],
                                    pa[:, 512 * h:512 * (h + 1)],
                                    start=(tb == 0), stop=(tb == NTB - 1))

                        # software-pipelined: scores+exp(tb) issue before av(tb-1)
                        prev = None
                        for tb in range(NTB):
                            tcol = b * S + tb * 128
                            sco = pssc.tile([128, 1024], F32, tag="score")
                            for h, kt in ((0, kT2a), (1, kT2b)):
                                nc.tensor.matmul(
                                    sco[:, 512 * h:512 * (h + 1)],
                                    kt[:, tcol:tcol + 128],
                                    qT[:, qcol:qcol + 512],
                                    start=True, stop=True)
                            pa = ptp.tile([128, 1024], F32R, tag="pt")
                            nc.scalar.activation(pa[:], sco[:], EXP, scale=0.125)
                            if prev is not None:
                                av_mms(*prev)
                            prev = (tb, pa)
                        av_mms(*prev)

                        a2 = []
                        for h in range(2):
                            s_t = mrgs.tile([128, 512], F32, tag="s_t")
                            nc.gpsimd.memset(s_t[64:128, :], 0.0)
                            nc.vector.tensor_copy(s_t[0:65, :], gam[h][0:65, :])
                            # 1/l: bounce the 512-long l row through DRAM into
                            # [128, 4] so DVE reciprocal runs 4 elems/lane,
                            # then broadcast over partitions on GpSimd.
                            lscr = drp.tile([512], F32, tag="lscr")
                            nc.sync.dma_start(out=lscr[None, :], in_=s_t[64:65, :])
                            l4 = ltp.tile([128, 4], F32, tag="l4")
                            nc.sync.dma_start(
                                out=l4[:],
                                in_=lscr[:].rearrange("(p f) -> p f", p=128))
                            r4 = ltp.tile([128, 4], F32, tag="r4")
                            nc.vector.reciprocal(r4[:], l4[:])
                            rscr = drp.tile([512], F32, tag="rscr")
                            nc.sync.dma_start(
                                out=rscr[:].rearrange("(p f) -> p f", p=128),
                                in_=r4[:])
                            rl1 = ltp.tile([128, 512], F32, tag="rl1")
                            nc.sync.dma_start(out=rl1[0:1, :], in_=rscr[None, :])
                            rlb = ltp.tile([128, 512], F32, tag="rlb")
                            nc.gpsimd.partition_broadcast(
                                out_ap=rlb[:], in_ap=rl1[0:1, :])
                            a2t = a2tp.tile([128, 512], F32R, tag="a2t")
                            nc.vector.tensor_tensor(
                                out=a2t[:], in0=s_t[:], in1=rlb[:], op=MUL)
                            a2.append(a2t)
                        for nb in range(4):
                            o = osbp.tile([128, 1024], F32, tag="osb")
                            for jc in range(2):
                                om = psmisc.tile([128, 512], F32, tag="misc")
                                nc.tensor.matmul(
                                    om[:], a2[0][:, nb * 128:(nb + 1) * 128],
                                    Wo_r[:, jc * 512:(jc + 1) * 512],
                                    start=True, stop=False)
                                nc.tensor.matmul(
                                    om[:], a2[1][:, nb * 128:(nb + 1) * 128],
                                    Wo_r[:, 1024 + jc * 512:1024 + (jc + 1) * 512],
                                    start=False, stop=True)
                                nc.vector.tensor_copy(
                                    o[:, jc * 512:(jc + 1) * 512], om[:])
                            nc.sync.dma_start(
                                out=out_d[qcol + nb * 128:qcol + (nb + 1) * 128, :],
                                in_=o[:])
    nc.compile()
    return nc


_PROG = None


def _get_program():
    global _PROG
    if _PROG is None:
        _PROG = _build_program()
    return _PROG


def _rope_tables():
    inv_freq = (1.0 / (10000.0 ** (np.arange(0, DH, 2, dtype=np.float32) / DH)))
    invf2 = inv_freq[np.arange(128) % 32]
    ang = np.arange(S, dtype=np.float32)[None, :] * invf2[:, None].astype(np.float32)
    return (np.cos(ang).astype(np.float32), np.sin(ang).astype(np.float32))


def make_in_maps(x, W_qkv, b_qkv, W_out, b_out):
    x = np.asarray(x, dtype=np.float32)
    W_qkv = np.asarray(W_qkv, dtype=np.float32)
    b_qkv = np.asarray(b_qkv, dtype=np.float32)
    W_out = np.asarray(W_out, dtype=np.float32)

    xT = np.ascontiguousarray(x.reshape(S2, D).T)
    ct, st = _rope_tables()

    in_maps = []
    for c in range(8):
        hA, hB = 2 * c, 2 * c + 1
        cols = np.r_[hA * DH:(hA + 1) * DH, hB * DH:(hB + 1) * DH]
        Wc = np.ascontiguousarray(
            np.concatenate([W_qkv[:, off + cols] for off in (0, D, 2 * D)], axis=1))
        bqc = np.ascontiguousarray(
            np.concatenate([b_qkv[off + cols] for off in (0, D, 2 * D)]))
        Woc = np.zeros((128, 2048), dtype=np.float32)
        Woc[0:64, 0:1024] = W_out[c * 128:c * 128 + 64, :]
        Woc[0:64, 1024:2048] = W_out[c * 128 + 64:(c + 1) * 128, :]
        in_maps.append(
            {"xT": xT, "W": Wc, "bq": bqc, "Wo": Woc, "ctab": ct, "stab": st})
    return in_maps


def assemble_output(results, b_out):
    acc = results[0]["out"].astype(np.float64)
    for c in range(1, 8):
        acc += results[c]["out"]
    out = acc + np.asarray(b_out, dtype=np.float64)
    return out.reshape(B, S, D).astype(np.float32)


def kernel(x, W_qkv, b_qkv, W_out, b_out):
    nc = _get_program()
    in_maps = make_in_maps(x, W_qkv, b_qkv, W_out, b_out)
    res = run_bass_kernel_spmd(nc, in_maps, core_ids=list(range(8)))
    return assemble_output(res.results, b_out)


if __name__ == "__main__":
    rng = np.random.default_rng(0)
    ins = {
        "x": rng.standard_normal((B, S, D), dtype=np.float32),
        "W_qkv": rng.standard_normal((D, 3 * D), dtype=np.float32) / 32.0,
        "b_qkv": np.zeros(3 * D, np.float32),
        "W_out": rng.standard_normal((D, D), dtype=np.float32) / 32.0,
        "b_out": np.zeros(D, np.float32),
    }
    o = kernel(**ins)
    print("kernel ran:", o.shape, o.dtype)


# revision 11
# speedup vs baseline: 1.0077x; 1.0077x over previous
"""Multi-head attention (B=2, S=2048, D=1024, H=16, RoPE) on 8 Trainium2 cores.

Sharding: tensor-parallel over heads. Core c owns heads (2c, 2c+1):
 - W_qkv column-sliced to that head pair (q|k|v blocks of 128 cols each),
 - W_out row-sliced to the pair's 128 input dims,
 - every core reads all tokens (x shipped pre-transposed as x^T),
 - each core emits a partial [4096, 1024] output; host sums the 8 partials
   and adds b_out (the Megatron-style allreduce done on host).

Device program (per core, identical SPMD):
  Phase A (128x128 PE mode): qkv^T = W_c^T @ x^T in fp32r, bias via a
    ones-row matmul, RoPE = (P2^T q^T) * sin + q^T * cos where P2 is the
    rotate-half +/-1 permutation (built on device), V transposed via PE
    into [t, d] layout with an appended ones column.
  Phase B (64x128 row-tiled PE): per (batch, 512-query chunk): for each
    128-key block: scores^T for both heads concurrently (head A on PE
    rows 0-63, head B on rows 64-127), exp on ACT (1/8 scale folded in),
    attn@V accumulated in PSUM with the ones column yielding softmax
    denominators for free; then merge/divide (reciprocal + broadcast
    matmul) and the output projection.
"""

import sys

if "/opt/trn_rl_repo" not in sys.path:
    sys.path.insert(0, "/opt/trn_rl_repo")

import numpy as np

import concourse.bacc as bacc
import concourse.mybir as mybir
from concourse import masks
from concourse.tile import TileContext
from concourse.bass_utils import run_bass_kernel_spmd

import os

F32 = mybir.dt.float32
F32R = mybir.dt.float32r
BF16 = mybir.dt.bfloat16
DT = BF16 if os.environ.get("MM_DT", "f32r") == "bf16" else F32R
ADD = mybir.AluOpType.add
MUL = mybir.AluOpType.mult
EXP = mybir.ActivationFunctionType.Exp

B, S, D, H, DH = 2, 2048, 1024, 16, 64
S2 = B * S              # 4096 tokens total
CH = 512                # Phase-A token chunk
NCH = S2 // CH          # 16 chunks
CPB = S // CH           # 8 chunks per batch
NSC = 4                 # 512-query chunks per batch
NTB = S // 128          # 16 key blocks per batch
VG = 130                # V2 group width: dA(64) | 1 | dB(64) | 1


def _build_program():
    nc = bacc.Bacc("TRN2", target_bir_lowering=False, debug=False, num_devices=8)

    xT = nc.dram_tensor("xT", [D, S2], F32, kind="ExternalInput")
    W = nc.dram_tensor("W", [D, 384], F32, kind="ExternalInput")
    bq = nc.dram_tensor("bq", [384], F32, kind="ExternalInput")
    Wo = nc.dram_tensor("Wo", [128, 2048], F32, kind="ExternalInput")
    ctab_d = nc.dram_tensor("ctab", [128, S], F32, kind="ExternalInput")
    stab_d = nc.dram_tensor("stab", [128, S], F32, kind="ExternalInput")
    out_d = nc.dram_tensor("out", [S2, D], F32, kind="ExternalOutput")

    xT_re = xT.rearrange("(kb p) n -> p kb n", p=128)   # [128, 8, 4096]
    W_re = W.rearrange("(kb p) m -> p kb m", p=128)     # [128, 8, 384]

    with TileContext(nc) as tc:
        with tc.tile_pool(name="consts", bufs=1) as cp:
            ident = cp.tile([128, 128], F32, tag="ident")
            masks.make_identity(nc, ident[:])

            P2r = cp.tile([128, 128], DT, tag="P2r")
            ones_r = cp.tile([128, CH], DT, tag="ones_r")     # row 0 = 1
            onesv = cp.tile([128, 64], F32, tag="onesv")        # all ones
            bq_r = cp.tile([128, 384], DT, tag="bq_r")        # row 0 = bias
            ctab = cp.tile([128, S], F32, tag="ctab")
            stab = cp.tile([128, S], F32, tag="stab")
            W_r = cp.tile([128, 8 * 384], DT, tag="W_r")
            Wo_r = cp.tile([128, 2048], DT, tag="Wo_r")
            qT = cp.tile([128, S2], DT, tag="qT")
            kT2a = cp.tile([128, S2], DT, tag="kT2a")  # rows 64:128 zero
            kT2b = cp.tile([128, S2], DT, tag="kT2b")  # rows 0:64 zero
            V2 = cp.tile([128, 2 * NTB * VG], DT, tag="V2")

            nc.sync.dma_start(out=ctab[:], in_=ctab_d[:])
            nc.sync.dma_start(out=stab[:], in_=stab_d[:])

            with tc.tile_pool(name="staging", bufs=1) as sp:
                # rotate-half matrix: P2[k, k^32] = -1 if (k%64)>=32 else +1
                p2f = sp.tile([128, 128], F32, tag="p2f")
                nc.gpsimd.memset(p2f[:], 0.0)
                for bk in (0, 64):
                    nc.gpsimd.affine_select(
                        out=p2f[bk:bk + 32, :], in_=p2f[bk:bk + 32, :],
                        compare_op=mybir.AluOpType.not_equal, fill=1.0,
                        base=bk + 32, channel_multiplier=1, pattern=[[-1, 128]])
                    nc.gpsimd.affine_select(
                        out=p2f[bk + 32:bk + 64, :], in_=p2f[bk + 32:bk + 64, :],
                        compare_op=mybir.AluOpType.not_equal, fill=-1.0,
                        base=bk, channel_multiplier=1, pattern=[[-1, 128]])
                nc.vector.tensor_copy(P2r[:], p2f[:])

                onesf = sp.tile([128, CH], F32, tag="onesf")
                nc.gpsimd.memset(onesf[:], 0.0)
                nc.gpsimd.memset(onesf[0:1, :], 1.0)
                nc.vector.tensor_copy(ones_r[:], onesf[:])

                nc.gpsimd.memset(onesv[:], 1.0)

                bqf = sp.tile([128, 384], F32, tag="bqf")
                nc.gpsimd.memset(bqf[:], 0.0)
                nc.sync.dma_start(out=bqf[0:1, :], in_=bq[None, :])
                nc.vector.tensor_copy(bq_r[:], bqf[:])

                wf = sp.tile([128, 8 * 384], F32, tag="wf")
                nc.sync.dma_start(
                    out=wf[:].rearrange("p (kb m) -> p kb m", kb=8), in_=W_re[:])
                nc.vector.tensor_copy(W_r[:], wf[:])

                wof = sp.tile([128, 2048], F32, tag="wof")
                nc.sync.dma_start(out=wof[:], in_=Wo[:])
                nc.vector.tensor_copy(Wo_r[:], wof[:])

                # zero halves of the padded K^T tiles
                zf = sp.tile([128, 512], F32, tag="zf")
                nc.gpsimd.memset(zf[:], 0.0)
                for i in range(8):
                    nc.vector.tensor_copy(
                        kT2a[64:128, i * 512:(i + 1) * 512], zf[64:128, :])
                    nc.vector.tensor_copy(
                        kT2b[0:64, i * 512:(i + 1) * 512], zf[0:64, :])

            # ones columns of V2 (cols 64 and 129 of each group)
            v2ones = V2[:].rearrange("p (g h r) -> p g h r", g=2 * NTB, h=2)
            nc.vector.tensor_copy(
                v2ones[:, :, :, 64:65],
                onesv[:].rearrange("p (g h r) -> p g h r", g=2 * NTB, h=2))

            # ---------------- Phase A: qkv + rope + V transpose ----------
            with tc.tile_pool(name="xc", bufs=2) as xcp, \
                 tc.tile_pool(name="xr", bufs=2) as xrp, \
                 tc.tile_pool(name="pre", bufs=4) as prep, \
                 tc.tile_pool(name="tmp", bufs=6) as tmpp, \
                 tc.tile_pool(name="vtc", bufs=2) as vtcp, \
                 tc.tile_pool(name="psqkv", bufs=4, space="PSUM") as psqkv, \
                 tc.tile_pool(name="psrot", bufs=2, space="PSUM") as psrot, \
                 tc.tile_pool(name="psv2", bufs=2, space="PSUM") as psv2:
                for ch in range(NCH):
                    cb = ch % CPB           # chunk index within batch
                    scol = cb * CH          # s-offset within batch
                    xc = xcp.tile([128, 8 * CH], F32, tag="xc")
                    nc.sync.dma_start(
                        out=xc[:].rearrange("p (kb n) -> p kb n", kb=8),
                        in_=xT_re[:, :, ch * CH:(ch + 1) * CH])
                    xr = xrp.tile([128, 8 * CH], DT, tag="xr")
                    nc.vector.tensor_copy(xr[:], xc[:])

                    ps3 = []
                    for mt in range(3):     # q, k, v
                        ps = psqkv.tile([128, CH], F32, tag="qkv")
                        for kb in range(8):
                            nc.tensor.matmul(
                                ps[:],
                                W_r[:, kb * 384 + mt * 128:kb * 384 + (mt + 1) * 128],
                                xr[:, kb * CH:(kb + 1) * CH],
                                start=(kb == 0), stop=False)
                        nc.tensor.matmul(      # bias: bq_pad^T @ ones-row
                            ps[:], bq_r[:, mt * 128:(mt + 1) * 128], ones_r[:],
                            start=False, stop=True)
                        ps3.append(ps)

                    # rope for q and k
                    for mt, dst in ((0, qT), (1, None)):
                        pre = prep.tile([128, CH], DT, tag="pre")
                        nc.vector.tensor_copy(pre[:], ps3[mt][:])
                        rot = psrot.tile([128, CH], F32, tag="rot")
                        nc.tensor.matmul(rot[:], P2r[:], pre[:], start=True, stop=True)
                        t1 = tmpp.tile([128, CH], F32, tag="tmp")
                        nc.vector.tensor_tensor(
                            out=t1[:], in0=rot[:],
                            in1=stab[:, scol:scol + CH], op=MUL)
                        t2 = tmpp.tile([128, CH], F32, tag="tmp")
                        nc.vector.tensor_tensor(
                            out=t2[:], in0=pre[:],
                            in1=ctab[:, scol:scol + CH], op=MUL)
                        if dst is not None:
                            nc.vector.tensor_tensor(
                                out=dst[:, ch * CH:(ch + 1) * CH],
                                in0=t1[:], in1=t2[:], op=ADD)
                        else:
                            nc.vector.tensor_tensor(
                                out=kT2a[0:64, ch * CH:(ch + 1) * CH],
                                in0=t1[0:64, :], in1=t2[0:64, :], op=ADD)
                            nc.vector.tensor_tensor(
                                out=kT2b[64:128, ch * CH:(ch + 1) * CH],
                                in0=t1[64:128, :], in1=t2[64:128, :], op=ADD)

                    # V: evac + PE transpose into [t, d] with ones cols
                    vt = vtcp.tile([128, CH], F32, tag="vt")
                    nc.vector.tensor_copy(vt[:], ps3[2][:])
                    nt = CH // 128
                    v2p = psv2.tile([128, CH], F32, tag="v2p")
                    for i in range(nt):
                        nc.tensor.transpose(
                            v2p[:, i * 128:(i + 1) * 128],
                            vt[:, i * 128:(i + 1) * 128], ident[:])
                    g0 = (ch // CPB) * NTB + cb * nt
                    dst = V2[:, g0 * VG:(g0 + nt) * VG].rearrange(
                        "p (i h r) -> p i h r", i=nt, h=2)[:, :, :, 0:64]
                    nc.vector.tensor_copy(
                        dst, v2p[:].rearrange("p (i h d) -> p i h d", i=nt, h=2))

            # ---------------- Phase B: attention + out projection --------
            with tc.tile_pool(name="pt", bufs=4) as ptp, \
                 tc.tile_pool(name="mrgc", bufs=2) as mrgc, \
                 tc.tile_pool(name="mrgs", bufs=4) as mrgs, \
                 tc.tile_pool(name="lt", bufs=4) as ltp, \
                 tc.tile_pool(name="a2t", bufs=4) as a2tp, \
                 tc.tile_pool(name="osb", bufs=2) as osbp, \
                 tc.tile_pool(name="dram", bufs=4, space="DRAM") as drp, \
                 tc.tile_pool(name="psscore", bufs=2, space="PSUM") as pssc, \
                 tc.tile_pool(name="psacc", bufs=2, space="PSUM") as psacc, \
                 tc.tile_pool(name="psmisc", bufs=2, space="PSUM") as psmisc:
                pending_out = []
                for b in range(B):
                    for sc in range(NSC):
                        qcol = b * S + sc * 512
                        gam = [psacc.tile([128, 512], F32, tag="acc",
                                          name=f"gam{i}")
                               for i in range(2)]

                        def av_mms(tb, pa):
                            gcol = (b * NTB + tb) * VG
                            for h in range(2):
                                nc.tensor.matmul(
                                    gam[h][0:65, :],
                                    V2[:, gcol + 65 * h:gcol + 65 * h + 65],
                                    pa[:, 512 * h:512 * (h + 1)],
                                    start=(tb == 0), stop=(tb == NTB - 1))

                        # software-pipelined: scores+exp(tb) issue before
                        # av(tb-1); previous sc's outproj mms drip in between
                        prev = None
                        for tb in range(NTB):
                            tcol = b * S + tb * 128
                            sco = pssc.tile([128, 1024], F32, tag="score")
                            for h, kt in ((0, kT2a), (1, kT2b)):
                                nc.tensor.matmul(
                                    sco[:, 512 * h:512 * (h + 1)],
                                    kt[:, tcol:tcol + 128],
                                    qT[:, qcol:qcol + 512],
                                    start=True, stop=True)
                            pa = ptp.tile([128, 1024], DT, tag="pt")
                            nc.scalar.activation(pa[:], sco[:], EXP, scale=0.125)
                            if prev is not None:
                                av_mms(*prev)
                            if pending_out:
                                pending_out.pop(0)()
                            prev = (tb, pa)
                        av_mms(*prev)

                        a2 = []
                        for h in range(2):
                            s_t = mrgs.tile([128, 512], F32, tag="s_t")
                            nc.gpsimd.memset(s_t[64:128, :], 0.0)
                            nc.vector.tensor_copy(s_t[0:65, :], gam[h][0:65, :])
                            # 1/l: bounce the 512-long l row through DRAM into
                            # [128, 4] so DVE reciprocal runs 4 elems/lane,
                            # then broadcast over partitions on GpSimd.
                            lscr = drp.tile([512], F32, tag="lscr")
                            nc.sync.dma_start(out=lscr[None, :], in_=s_t[64:65, :])
                            l4 = ltp.tile([128, 4], F32, tag="l4")
                            nc.sync.dma_start(
                                out=l4[:],
                                in_=lscr[:].rearrange("(p f) -> p f", p=128))
                            r4 = ltp.tile([128, 4], F32, tag="r4")
                            nc.vector.reciprocal(r4[:], l4[:])
                            rscr = drp.tile([512], F32, tag="rscr")
                            nc.sync.dma_start(
                                out=rscr[:].rearrange("(p f) -> p f", p=128),
                                in_=r4[:])
                            rl1 = ltp.tile([128, 512], F32, tag="rl1")
                            nc.sync.dma_start(out=rl1[0:1, :], in_=rscr[None, :])
                            rlb = ltp.tile([128, 512], F32, tag="rlb")
                            nc.gpsimd.partition_broadcast(
                                out_ap=rlb[:], in_ap=rl1[0:1, :])
                            a2t = a2tp.tile([128, 512], DT, tag="a2t")
                            nc.vector.tensor_tensor(
                                out=a2t[:], in0=s_t[:], in1=rlb[:], op=MUL)
                            a2.append(a2t)
                        def make_outproj(qcol, a2):
                            def emit_nb(nb):
                                o = osbp.tile([128, 1024], F32, tag="osb",
                                              name=f"osb{qcol}_{nb}")
                                for jc in range(2):
                                    om = psmisc.tile([128, 512], F32, tag="misc",
                                                     name=f"om{qcol}_{nb}_{jc}")
                                    nc.tensor.matmul(
                                        om[:], a2[0][:, nb * 128:(nb + 1) * 128],
                                        Wo_r[:, jc * 512:(jc + 1) * 512],
                                        start=True, stop=False)
                                    nc.tensor.matmul(
                                        om[:], a2[1][:, nb * 128:(nb + 1) * 128],
                                        Wo_r[:, 1024 + jc * 512:1024 + (jc + 1) * 512],
                                        start=False, stop=True)
                                    nc.vector.tensor_copy(
                                        o[:, jc * 512:(jc + 1) * 512], om[:])
                                nc.sync.dma_start(
                                    out=out_d[qcol + nb * 128:
                                              qcol + (nb + 1) * 128, :],
                                    in_=o[:])
                            return [lambda nb=nb: emit_nb(nb) for nb in range(4)]

                        pending_out.extend(make_outproj(qcol, a2))
                for fn in pending_out:
                    fn()
    nc.compile()
    return nc


_PROG = None


def _get_program():
    global _PROG
    if _PROG is None:
        _PROG = _build_program()
    return _PROG


def _rope_tables():
    inv_freq = (1.0 / (10000.0 ** (np.arange(0, DH, 2, dtype=np.float32) / DH)))
    invf2 = inv_freq[np.arange(128) % 32]
    ang = np.arange(S, dtype=np.float32)[None, :] * invf2[:, None].astype(np.float32)
    return (np.cos(ang).astype(np.float32), np.sin(ang).astype(np.float32))


def make_in_maps(x, W_qkv, b_qkv, W_out, b_out):
    x = np.asarray(x, dtype=np.float32)
    W_qkv = np.asarray(W_qkv, dtype=np.float32)
    b_qkv = np.asarray(b_qkv, dtype=np.float32)
    W_out = np.asarray(W_out, dtype=np.float32)

    xT = np.ascontiguousarray(x.reshape(S2, D).T)
    ct, st = _rope_tables()

    in_maps = []
    for c in range(8):
        hA, hB = 2 * c, 2 * c + 1
        cols = np.r_[hA * DH:(hA + 1) * DH, hB * DH:(hB + 1) * DH]
        Wc = np.ascontiguousarray(
            np.concatenate([W_qkv[:, off + cols] for off in (0, D, 2 * D)], axis=1))
        bqc = np.ascontiguousarray(
            np.concatenate([b_qkv[off + cols] for off in (0, D, 2 * D)]))
        Woc = np.zeros((128, 2048), dtype=np.float32)
        Woc[0:64, 0:1024] = W_out[c * 128:c * 128 + 64, :]
        Woc[0:64, 1024:2048] = W_out[c * 128 + 64:(c + 1) * 128, :]
        in_maps.append(
            {"xT": xT, "W": Wc, "bq": bqc, "Wo": Woc, "ctab": ct, "stab": st})
    return in_maps


def assemble_output(results, b_out):
    acc = results[0]["out"].astype(np.float64)
    for c in range(1, 8):
        acc += results[c]["out"]
    out = acc + np.asarray(b_out, dtype=np.float64)
    return out.reshape(B, S, D).astype(np.float32)


def kernel(x, W_qkv, b_qkv, W_out, b_out):
    nc = _get_program()
    in_maps = make_in_maps(x, W_qkv, b_qkv, W_out, b_out)
    res = run_bass_kernel_spmd(nc, in_maps, core_ids=list(range(8)))
    return assemble_output(res.results, b_out)


if __name__ == "__main__":
    rng = np.random.default_rng(0)
    ins = {
        "x": rng.standard_normal((B, S, D), dtype=np.float32),
        "W_qkv": rng.standard_normal((D, 3 * D), dtype=np.float32) / 32.0,
        "b_qkv": np.zeros(3 * D, np.float32),
        "W_out": rng.standard_normal((D, D), dtype=np.float32) / 32.0,
        "b_out": np.zeros(D, np.float32),
    }
    o = kernel(**ins)
    print("kernel ran:", o.shape, o.dtype)


# revision 14
# speedup vs baseline: 1.0864x; 1.0780x over previous
"""Multi-head attention (B=2, S=2048, D=1024, H=16, RoPE) on 8 Trainium2 cores.

Sharding: tensor-parallel over heads. Core c owns heads (2c, 2c+1):
 - W_qkv column-sliced to that head pair (q|k|v blocks of 128 cols each),
 - W_out row-sliced to the pair's 128 input dims,
 - every core reads all tokens (x shipped pre-transposed as x^T),
 - each core emits a partial [4096, 1024] output; host sums the 8 partials
   and adds b_out (the Megatron-style allreduce done on host).

Device program (per core, identical SPMD, all matmuls in fp32r and plain
128x128 PE mode):
  Per chunk of 256 tokens: qkv^T = W_c^T @ x^T, bias via a ones-row
  matmul, RoPE = (P2^T q^T) * sin + q^T * cos with P2 the rotate-half
  +/-1 permutation built on device; V transposed via PE into [t, d]
  layout. K^T is stored zero-padded per head so score matmuls run with
  K=128 and no PE mode switches.
  Attention per (batch, 512-query chunk), software-pipelined over
  128-key blocks: scores^T for both heads into one 2-bank PSUM tile,
  one batched exp on ACT (1/8 scale folded in), attn@V accumulated in
  PSUM. The V layout [V_A | 1 | zeros | 1 | V_B] makes head A land on
  PSUM rows 0-63 and head B on rows 64-127 with softmax denominators in
  rows 64/63 for free, so the divided attention output assembles into
  one dense [128, n] operand and the output projection is a single
  K=128 matmul per tile. 1/l is computed by bouncing the 512-long l row
  through DRAM into [128, 4] (reciprocal runs 4 elems/lane), broadcast
  back over partitions on GpSimd. Batch 1's projections interleave with
  batch 0's attention.
"""

import os
import sys

if "/opt/trn_rl_repo" not in sys.path:
    sys.path.insert(0, "/opt/trn_rl_repo")

import numpy as np

import concourse.bacc as bacc
import concourse.mybir as mybir
from concourse import masks
from concourse.tile import TileContext
from concourse.bass_utils import run_bass_kernel_spmd

F32 = mybir.dt.float32
F32R = mybir.dt.float32r
BF16 = mybir.dt.bfloat16
DT = BF16 if os.environ.get("MM_DT", "f32r") == "bf16" else F32R
ADD = mybir.AluOpType.add
MUL = mybir.AluOpType.mult
EXP = mybir.ActivationFunctionType.Exp

B, S, D, H, DH = 2, 2048, 1024, 16, 64
S2 = B * S              # 4096 tokens total
CH = 256                # token chunk for the projection phase
CPB = S // CH           # 8 chunks per batch
NSC = 4                 # 512-query chunks per batch
NTB = S // 128          # 16 key blocks per batch
VG = 193                # V2 group: VA(64) | 1 | zeros(63) | 1 | VB(64)


def _build_program():
    nc = bacc.Bacc("TRN2", target_bir_lowering=False, debug=False, num_devices=8)

    xT = nc.dram_tensor("xT", [D, S2], F32, kind="ExternalInput")
    W = nc.dram_tensor("W", [D, 384], F32, kind="ExternalInput")
    bq = nc.dram_tensor("bq", [384], F32, kind="ExternalInput")
    Wo = nc.dram_tensor("Wo", [128, 1024], F32, kind="ExternalInput")
    ctab_d = nc.dram_tensor("ctab", [128, S], F32, kind="ExternalInput")
    stab_d = nc.dram_tensor("stab", [128, S], F32, kind="ExternalInput")
    out_d = nc.dram_tensor("out", [S2, D], F32, kind="ExternalOutput")

    xT_re = xT.rearrange("(kb p) n -> p kb n", p=128)   # [128, 8, 4096]
    W_re = W.rearrange("(kb p) m -> p kb m", p=128)     # [128, 8, 384]

    with TileContext(nc) as tc:
        with tc.tile_pool(name="consts", bufs=1) as cp, \
             tc.tile_pool(name="xc", bufs=2) as xcp, \
             tc.tile_pool(name="xr", bufs=2) as xrp, \
             tc.tile_pool(name="pre", bufs=4) as prep, \
             tc.tile_pool(name="tmp", bufs=4) as tmpp, \
             tc.tile_pool(name="vtc", bufs=2) as vtcp, \
             tc.tile_pool(name="pt", bufs=3) as ptp, \
             tc.tile_pool(name="mrgs", bufs=3) as mrgs, \
             tc.tile_pool(name="lt", bufs=2) as ltp, \
             tc.tile_pool(name="a2c", bufs=2) as a2cp, \
             tc.tile_pool(name="osb", bufs=2) as osbp, \
             tc.tile_pool(name="dram", bufs=4, space="DRAM") as drp, \
             tc.tile_pool(name="ps512", bufs=2, space="PSUM") as ps512, \
             tc.tile_pool(name="psscore", bufs=2, space="PSUM") as pssc, \
             tc.tile_pool(name="psacc", bufs=1, space="PSUM") as psacc:
            ident = cp.tile([128, 128], F32, tag="ident")
            masks.make_identity(nc, ident[:])

            P2r = cp.tile([128, 128], DT, tag="P2r")
            ones_r = cp.tile([128, CH], DT, tag="ones_r")       # row 0 = 1
            onesv = cp.tile([128, 64], F32, tag="onesv")        # all ones
            bq_r = cp.tile([128, 384], DT, tag="bq_r")          # row 0 = bias
            ctab = cp.tile([128, S], F32, tag="ctab")
            stab = cp.tile([128, S], F32, tag="stab")
            W_r = cp.tile([128, 8 * 384], DT, tag="W_r")
            Wo_r = cp.tile([128, 1024], DT, tag="Wo_r")
            qTb = [cp.tile([128, S], DT, name=f"qT{b}", tag=f"qT{b}")
                   for b in range(B)]
            kTab = [cp.tile([128, S], DT, name=f"kTa{b}", tag=f"kTa{b}")
                    for b in range(B)]   # head A rows 0:64, rows 64:128 zero
            kTbb = [cp.tile([128, S], DT, name=f"kTb{b}", tag=f"kTb{b}")
                    for b in range(B)]   # head B rows 64:128, rows 0:64 zero
            V2b = [cp.tile([128, NTB * VG], DT, name=f"V2{b}", tag=f"V2{b}")
                   for b in range(B)]

            nc.sync.dma_start(out=ctab[:], in_=ctab_d[:])
            nc.sync.dma_start(out=stab[:], in_=stab_d[:])

            # setup staging reuses the chunk pools (no extra SBUF)
            # rotate-half matrix: P2[k, k^32] = -1 if (k%64)>=32 else +1
            p2f = tmpp.tile([128, 128], F32, tag="tmp", name="p2f")
            nc.gpsimd.memset(p2f[:], 0.0)
            for bk in (0, 64):
                nc.gpsimd.affine_select(
                    out=p2f[bk:bk + 32, :], in_=p2f[bk:bk + 32, :],
                    compare_op=mybir.AluOpType.not_equal, fill=1.0,
                    base=bk + 32, channel_multiplier=1, pattern=[[-1, 128]])
                nc.gpsimd.affine_select(
                    out=p2f[bk + 32:bk + 64, :], in_=p2f[bk + 32:bk + 64, :],
                    compare_op=mybir.AluOpType.not_equal, fill=-1.0,
                    base=bk, channel_multiplier=1, pattern=[[-1, 128]])
            nc.vector.tensor_copy(P2r[:], p2f[:])

            onesf = tmpp.tile([128, CH], F32, tag="tmp", name="onesf")
            nc.gpsimd.memset(onesf[:], 0.0)
            nc.gpsimd.memset(onesf[0:1, :], 1.0)
            nc.vector.tensor_copy(ones_r[:], onesf[:])

            nc.gpsimd.memset(onesv[:], 1.0)

            bqf = tmpp.tile([128, 384], F32, tag="tmp", name="bqf")
            nc.gpsimd.memset(bqf[:], 0.0)
            nc.sync.dma_start(out=bqf[0:1, :], in_=bq[None, :])
            nc.vector.tensor_copy(bq_r[:], bqf[:])

            for half in range(2):
                wf = xcp.tile([128, 4 * 384], F32, tag="xc", name=f"wf{half}")
                nc.sync.dma_start(
                    out=wf[:].rearrange("p (kb m) -> p kb m", kb=4),
                    in_=W_re[:, half * 4:(half + 1) * 4, :])
                nc.vector.tensor_copy(
                    W_r[:, half * 4 * 384:(half + 1) * 4 * 384], wf[:])

            wof = xcp.tile([128, 1024], F32, tag="xc", name="wof")
            nc.sync.dma_start(out=wof[:], in_=Wo[:])
            nc.vector.tensor_copy(Wo_r[:], wof[:])

            # zero pads of the per-head K^T tiles; V2 zero/ones columns
            zf = xcp.tile([128, 512], F32, tag="xc", name="zf")
            nc.gpsimd.memset(zf[:], 0.0)
            for b in range(B):
                for i in range(4):
                    sl = slice(i * 512, (i + 1) * 512)
                    nc.vector.tensor_copy(kTab[b][64:128, sl], zf[64:128, :])
                    nc.vector.tensor_copy(kTbb[b][0:64, sl], zf[0:64, :])
                for g in range(NTB):
                    nc.vector.tensor_copy(
                        V2b[b][:, g * VG + 65:g * VG + 128], zf[:, 0:63])
                v2o = V2b[b][:].rearrange("p (g c) -> p g c", g=NTB)
                ov = onesv[:].rearrange("p (g c) -> p g c", g=16)
                nc.vector.tensor_copy(v2o[:, :, 64:65], ov[:, :, 0:1])
                nc.vector.tensor_copy(v2o[:, :, 128:129], ov[:, :, 0:1])

            # ---------------- emitters ----------------------------------
            def emit_chunk(ch):
                bb, cb = ch // CPB, ch % CPB
                scol = cb * CH
                xc = xcp.tile([128, 8 * CH], F32, tag="xc", name=f"xc{ch}")
                nc.sync.dma_start(
                    out=xc[:].rearrange("p (kb n) -> p kb n", kb=8),
                    in_=xT_re[:, :, ch * CH:(ch + 1) * CH])
                xr = xrp.tile([128, 8 * CH], DT, tag="xr", name=f"xr{ch}")
                nc.vector.tensor_copy(xr[:], xc[:])

                ps3 = []
                for mt in range(3):     # q, k, v
                    ps = ps512.tile([128, CH], F32, tag="ps512",
                                    name=f"qkv{ch}_{mt}")
                    for kb in range(8):
                        nc.tensor.matmul(
                            ps[:],
                            W_r[:, kb * 384 + mt * 128:kb * 384 + (mt + 1) * 128],
                            xr[:, kb * CH:(kb + 1) * CH],
                            start=(kb == 0), stop=False)
                    nc.tensor.matmul(      # bias via ones row
                        ps[:], bq_r[:, mt * 128:(mt + 1) * 128], ones_r[:],
                        start=False, stop=True)
                    ps3.append(ps)

                # rope for q and k
                for mt in (0, 1):
                    pre = prep.tile([128, CH], DT, tag="pre", name=f"pre{ch}_{mt}")
                    nc.vector.tensor_copy(pre[:], ps3[mt][:])
                    rot = ps512.tile([128, CH], F32, tag="ps512",
                                     name=f"rot{ch}_{mt}")
                    nc.tensor.matmul(rot[:], P2r[:], pre[:], start=True, stop=True)
                    t1 = tmpp.tile([128, CH], F32, tag="tmp", name=f"t1_{ch}_{mt}")
                    nc.vector.tensor_tensor(
                        out=t1[:], in0=rot[:], in1=stab[:, scol:scol + CH], op=MUL)
                    t2 = tmpp.tile([128, CH], F32, tag="tmp", name=f"t2_{ch}_{mt}")
                    nc.vector.tensor_tensor(
                        out=t2[:], in0=pre[:], in1=ctab[:, scol:scol + CH], op=MUL)
                    csl = slice(scol, scol + CH)
                    if mt == 0:
                        nc.vector.tensor_tensor(
                            out=qTb[bb][:, csl], in0=t1[:], in1=t2[:], op=ADD)
                    else:
                        nc.vector.tensor_tensor(
                            out=kTab[bb][0:64, csl],
                            in0=t1[0:64, :], in1=t2[0:64, :], op=ADD)
                        nc.vector.tensor_tensor(
                            out=kTbb[bb][64:128, csl],
                            in0=t1[64:128, :], in1=t2[64:128, :], op=ADD)

                # V: evac + PE transpose into [t, d]
                nt = CH // 128
                vt = vtcp.tile([128, CH], F32, tag="vt", name=f"vt{ch}")
                nc.vector.tensor_copy(vt[:], ps3[2][:])
                v2p = ps512.tile([128, CH], F32, tag="ps512", name=f"v2p{ch}")
                for i in range(nt):
                    nc.tensor.transpose(
                        v2p[:, i * 128:(i + 1) * 128],
                        vt[:, i * 128:(i + 1) * 128], ident[:])
                g0 = cb * nt
                dst = V2b[bb][:, g0 * VG:(g0 + nt) * VG].rearrange(
                    "p (i c) -> p i c", i=nt)
                src = v2p[:].rearrange("p (i h d) -> p i h d", i=nt, h=2)
                # head A -> cols 0:64, head B -> cols 129:193 of each group
                nc.vector.tensor_copy(dst[:, :, 0:64], src[:, :, 0:1, :])
                nc.vector.tensor_copy(dst[:, :, 129:193], src[:, :, 1:2, :])

            pending_out = []

            def emit_sc(bb, sc):
                qcol = sc * 512
                qT, kTa, kTb, V2 = qTb[bb], kTab[bb], kTbb[bb], V2b[bb]
                gam = psacc.tile([128, 1024], F32, tag="acc",
                                 name=f"gam{bb}_{sc}")

                def av_mms(tb, pa):
                    gcol = tb * VG
                    nc.tensor.matmul(
                        gam[0:65, 0:512],
                        V2[:, gcol:gcol + 65], pa[:, 0:512],
                        start=(tb == 0), stop=(tb == NTB - 1))
                    nc.tensor.matmul(
                        gam[:, 512:1024],
                        V2[:, gcol + 65:gcol + 193], pa[:, 512:1024],
                        start=(tb == 0), stop=(tb == NTB - 1))

                prev = None
                for tb in range(NTB):
                    tcol = tb * 128
                    sco = pssc.tile([128, 1024], F32, tag="score",
                                    name=f"sco{bb}_{sc}_{tb}")
                    for h, kt in ((0, kTa), (1, kTb)):
                        nc.tensor.matmul(
                            sco[:, 512 * h:512 * (h + 1)],
                            kt[:, tcol:tcol + 128], qT[:, qcol:qcol + 512],
                            start=True, stop=True)
                    pa = ptp.tile([128, 1024], DT, tag="pt",
                                  name=f"pa{bb}_{sc}_{tb}")
                    nc.scalar.activation(pa[:], sco[:], EXP, scale=0.125)
                    if prev is not None:
                        av_mms(*prev)
                    if pending_out:
                        pending_out.pop(0)()
                    prev = (tb, pa)
                av_mms(*prev)

                # merge + divide: head A rows 0:63 (l at row 64 of gam-A),
                # head B rows 64:127 (l at row 63 of gam-B)
                a2 = a2cp.tile([128, 512], DT, tag="a2c", name=f"a2c{bb}_{sc}")
                for h in range(2):
                    s_t = mrgs.tile([128, 512], F32, tag="s_t",
                                    name=f"s_t{bb}_{sc}_{h}")
                    if h == 0:
                        nc.gpsimd.memset(s_t[64:128, :], 0.0)
                        nc.vector.tensor_copy(s_t[0:65, :], gam[0:65, 0:512])
                        lrow = s_t[64:65, :]
                    else:
                        nc.vector.tensor_copy(s_t[:], gam[:, 512:1024])
                        lrow = s_t[63:64, :]
                    lscr = drp.tile([512], F32, tag="lscr",
                                    name=f"ls{bb}_{sc}_{h}")
                    nc.sync.dma_start(out=lscr[None, :], in_=lrow)
                    l4 = ltp.tile([128, 4], F32, tag="l4", name=f"l4_{bb}{sc}{h}")
                    nc.sync.dma_start(
                        out=l4[:], in_=lscr[:].rearrange("(p f) -> p f", p=128))
                    r4 = ltp.tile([128, 4], F32, tag="r4", name=f"r4_{bb}{sc}{h}")
                    nc.vector.reciprocal(r4[:], l4[:])
                    rscr = drp.tile([512], F32, tag="rscr",
                                    name=f"rs{bb}_{sc}_{h}")
                    nc.sync.dma_start(
                        out=rscr[:].rearrange("(p f) -> p f", p=128), in_=r4[:])
                    rl1 = ltp.tile([128, 512], F32, tag="rl1",
                                   name=f"rl1_{bb}{sc}{h}")
                    nc.sync.dma_start(out=rl1[0:1, :], in_=rscr[None, :])
                    rlb = ltp.tile([128, 512], F32, tag="rlb",
                                   name=f"rlb_{bb}{sc}{h}")
                    nc.gpsimd.partition_broadcast(out_ap=rlb[:], in_ap=rl1[0:1, :])
                    if h == 0:
                        nc.vector.tensor_tensor(
                            out=a2[0:64, :], in0=s_t[0:64, :],
                            in1=rlb[0:64, :], op=MUL)
                    else:
                        nc.vector.tensor_tensor(
                            out=a2[64:128, :], in0=s_t[64:128, :],
                            in1=rlb[64:128, :], op=MUL)

                def make_outproj(bb, qcol, a2):
                    def emit_nb(nb):
                        o = osbp.tile([128, 1024], F32, tag="osb",
                                      name=f"osb{bb}_{qcol}_{nb}")
                        for jc in range(2):
                            om = ps512.tile([128, 512], F32, tag="ps512",
                                            name=f"om{bb}_{qcol}_{nb}_{jc}")
                            nc.tensor.matmul(
                                om[:], a2[:, nb * 128:(nb + 1) * 128],
                                Wo_r[:, jc * 512:(jc + 1) * 512],
                                start=True, stop=True)
                            nc.vector.tensor_copy(
                                o[:, jc * 512:(jc + 1) * 512], om[:])
                        nc.sync.dma_start(
                            out=out_d[bb * S + qcol + nb * 128:
                                      bb * S + qcol + (nb + 1) * 128, :],
                            in_=o[:])
                    return [lambda nb=nb: emit_nb(nb) for nb in range(4)]

                pending_out.extend(make_outproj(bb, qcol, a2))

            # ---------------- schedule ----------------------------------
            for ch in range(CPB):           # batch 0 projections
                emit_chunk(ch)
            for sc in range(NSC):           # batch 0 attention || batch 1 proj
                emit_sc(0, sc)
                emit_chunk(CPB + 2 * sc)
                emit_chunk(CPB + 2 * sc + 1)
            for sc in range(NSC):           # batch 1 attention
                emit_sc(1, sc)
            for fn in pending_out:
                fn()

    nc.compile()
    return nc


_PROG = None


def _get_program():
    global _PROG
    if _PROG is None:
        _PROG = _build_program()
    return _PROG


def _rope_tables():
    inv_freq = (1.0 / (10000.0 ** (np.arange(0, DH, 2, dtype=np.float32) / DH)))
    invf2 = inv_freq[np.arange(128) % 32]
    ang = np.arange(S, dtype=np.float32)[None, :] * invf2[:, None].astype(np.float32)
    return (np.cos(ang).astype(np.float32), np.sin(ang).astype(np.float32))


def make_in_maps(x, W_qkv, b_qkv, W_out, b_out):
    x = np.asarray(x, dtype=np.float32)
    W_qkv = np.asarray(W_qkv, dtype=np.float32)
    b_qkv = np.asarray(b_qkv, dtype=np.float32)
    W_out = np.asarray(W_out, dtype=np.float32)

    xT = np.ascontiguousarray(x.reshape(S2, D).T)
    ct, st = _rope_tables()

    in_maps = []
    for c in range(8):
        hA, hB = 2 * c, 2 * c + 1
        cols = np.r_[hA * DH:(hA + 1) * DH, hB * DH:(hB + 1) * DH]
        Wc = np.ascontiguousarray(
            np.concatenate([W_qkv[:, off + cols] for off in (0, D, 2 * D)], axis=1))
        bqc = np.ascontiguousarray(
            np.concatenate([b_qkv[off + cols] for off in (0, D, 2 * D)]))
        Woc = np.ascontiguousarray(W_out[c * 128:(c + 1) * 128, :])
        in_maps.append(
            {"xT": xT, "W": Wc, "bq": bqc, "Wo": Woc, "ctab": ct, "stab": st})
    return in_maps


def assemble_output(results, b_out):
    acc = results[0]["out"].astype(np.float64)
    for c in range(1, 8):
        acc += results[c]["out"]
    out = acc + np.asarray(b_out, dtype=np.float64)
    return out.reshape(B, S, D).astype(np.float32)


def kernel(x, W_qkv, b_qkv, W_out, b_out):
    nc = _get_program()
    in_maps = make_in_maps(x, W_qkv, b_qkv, W_out, b_out)
    res = run_bass_kernel_spmd(nc, in_maps, core_ids=list(range(8)))
    return assemble_output(res.results, b_out)


if __name__ == "__main__":
    rng = np.random.default_rng(0)
    ins = {
        "x": rng.standard_normal((B, S, D), dtype=np.float32),
        "W_qkv": rng.standard_normal((D, 3 * D), dtype=np.float32) / 32.0,
        "b_qkv": np.zeros(3 * D, np.float32),
        "W_out": rng.standard_normal((D, D), dtype=np.float32) / 32.0,
        "b_out": np.zeros(D, np.float32),
    }
    o = kernel(**ins)
    print("kernel ran:", o.shape, o.dtype)
